# revision 16
# baseline (speedup 1.0000x reference)
"""BiLSTM-CRF Trainium2 kernel (8-core SPMD, no collectives).

Strategy: chunk-parallel scans with warmup. The LSTM forget-gate contraction
(~0.6/step for these weights) makes a chunk that warms up for W steps from a
wrong initial state converge to the exact sequential state; chunks that cross
the true sequence boundary get an exact masked state override instead. The
same idea parallelizes the Viterbi forward pass (max-plus coalescing). Each
core independently processes a 1088-token slab (1024 tokens + halo for the
Viterbi warmup): embedding gather (indirect DMA) -> input projections (PE)
-> 57-step vectorized BiLSTM scan over 128 chunk-lanes -> emission matmul ->
40-step vectorized Viterbi with bulk backpointer extraction. The host only
does the O(T) backtrace pointer-chase and the scalar score sum.
"""

import numpy as np

import concourse.bass as bass
import concourse.tile as tile
from concourse import bacc, mybir
from concourse.bass import AP
from concourse.masks import make_identity

f32 = mybir.dt.float32
i32 = mybir.dt.int32
bf16 = mybir.dt.bfloat16

# problem constants (hardcoded per contract)
VOCAB = 1_000_000
EMB = 32
H2 = 128
G4 = 4 * H2
K = 11
START_IDX = 9
END_IDX = 10
T = 8192
NCORES = 8

# geometry
P = 128
NL = 64            # lanes per direction per core
L = 17             # real tokens per LSTM lane
W = 32             # LSTM warmup steps
SLAB = 1088        # NL * L, real tokens per core (incl. viterbi halo)
WIN = SLAB + 2 * W  # gathered token window (1168), padded to GPAD
GPAD = 1280        # 10 gather groups of 128
NSTEP = W + L      # 57
LV = 8             # real tokens per viterbi lane
WV = 32            # viterbi warmup steps
NVSTEP = WV + LV   # 40
BIG = 2048.0
NEG_INIT = -10000.0

# reset (exact-init override) steps
LRESET_F = [(W + 64 - 17 * l, l) for l in range(8) if 0 <= W + 64 - 17 * l <= NSTEP - 1]
LRESET_B = [(W - 17 * j, j) for j in range(8) if 0 <= W - 17 * j <= NSTEP - 1]
LRESET_STEPS = sorted({s for s, _ in LRESET_F} | {s for s, _ in LRESET_B})
VRESET = [(WV - LV * j, j) for j in range(WV // LV + 1) if WV - LV * j >= 0]
VRESET_STEPS = sorted({s for s, _ in VRESET})

_PROGRAM_CACHE = {}


def _ap(t_ap, extra_offset, dims):
    return AP(t_ap.tensor, t_ap.offset + extra_offset, [t_ap.ap[0]] + dims)


def build_program(debug=False):
    key = bool(debug)
    if key in _PROGRAM_CACHE:
        return _PROGRAM_CACHE[key]

    nc = bacc.Bacc("TRN2", target_bir_lowering=False, debug=True)

    d_table = nc.dram_tensor("table", (VOCAB, EMB), f32, kind="ExternalInput")
    d_tok = nc.dram_tensor("tok", (GPAD,), i32, kind="ExternalInput")
    d_tokr = nc.dram_tensor("tokr", (GPAD,), i32, kind="ExternalInput")
    d_whh_f_hi = nc.dram_tensor("whh_f_hi", (P, G4), bf16, kind="ExternalInput")
    d_whh_f_lo = nc.dram_tensor("whh_f_lo", (P, G4), bf16, kind="ExternalInput")
    d_whh_b_hi = nc.dram_tensor("whh_b_hi", (P, G4), bf16, kind="ExternalInput")
    d_whh_b_lo = nc.dram_tensor("whh_b_lo", (P, G4), bf16, kind="ExternalInput")
    d_wih_f = nc.dram_tensor("wih_f", (P, G4), f32, kind="ExternalInput")
    d_wih_b = nc.dram_tensor("wih_b", (P, G4), f32, kind="ExternalInput")
    d_wout = nc.dram_tensor("woutT", (P, 2 * K), f32, kind="ExternalInput")
    d_bout = nc.dram_tensor("bout", (K, 1), f32, kind="ExternalInput")
    d_transb = nc.dram_tensor("transb", (P, K * K), f32, kind="ExternalInput")
    n_lr = len(LRESET_STEPS)
    n_vr = len(VRESET_STEPS)
    d_lmask = nc.dram_tensor("lmask", (n_lr, 3, P, P), f32, kind="ExternalInput")
    d_vmask = nc.dram_tensor("vmask", (n_vr, 2, P, K), f32, kind="ExternalInput")

    d_bp = nc.dram_tensor("bp", (P, LV * K), i32, kind="ExternalOutput")
    d_fv = nc.dram_tensor("fv", (P, K), f32, kind="ExternalOutput")
    d_feats = nc.dram_tensor("feats", (K, SLAB), f32, kind="ExternalOutput")
    if debug:
        d_hf = nc.dram_tensor("hf", (P, SLAB), f32, kind="ExternalOutput")
        d_hb = nc.dram_tensor("hb", (P, SLAB), f32, kind="ExternalOutput")

    NG = GPAD // P  # gather groups

    with tile.TileContext(nc) as tc:
        with (
            tc.tile_pool(name="const", bufs=1) as cp,
            tc.tile_pool(name="big", bufs=1) as bigp,
            tc.tile_pool(name="gat", bufs=3) as gp,
            tc.tile_pool(name="state", bufs=3) as st,
            tc.tile_pool(name="work", bufs=3) as wk,
            tc.tile_pool(name="ps", bufs=2, space="PSUM") as pp,
            tc.tile_pool(name="psz", bufs=2, space="PSUM") as ppz,
            tc.tile_pool(name="psf", bufs=1, space="PSUM") as ppf,
        ):
            # ---- constants ----
            ident = cp.tile([P, P], f32)
            make_identity(nc, ident[:])
            whh_f_hi = cp.tile([P, G4], bf16)
            whh_f_lo = cp.tile([P, G4], bf16)
            whh_b_hi = cp.tile([P, G4], bf16)
            whh_b_lo = cp.tile([P, G4], bf16)
            wih_f = cp.tile([P, G4], f32)
            wih_b = cp.tile([P, G4], f32)
            wout = cp.tile([P, 2 * K], f32)
            bout = cp.tile([K, 1], f32)
            transb = cp.tile([P, K * K], f32)
            nc.sync.dma_start(whh_f_hi[:], d_whh_f_hi[:])
            nc.sync.dma_start(whh_f_lo[:], d_whh_f_lo[:])
            nc.sync.dma_start(whh_b_hi[:], d_whh_b_hi[:])
            nc.sync.dma_start(whh_b_lo[:], d_whh_b_lo[:])
            nc.sync.dma_start(wih_f[:], d_wih_f[:])
            nc.sync.dma_start(wih_b[:], d_wih_b[:])
            nc.sync.dma_start(wout[:], d_wout[:])
            nc.sync.dma_start(bout[:], d_bout[:])
            nc.sync.dma_start(transb[:], d_transb[:])

            # contiguous per-gate weight tiles so LDWEIGHTS can use FWL
            wg = {}
            for di, (whi_src, wlo_src) in enumerate(((whh_f_hi, whh_f_lo),
                                                    (whh_b_hi, whh_b_lo))):
                for gi in range(4):
                    thi = cp.tile([P, P], bf16, tag=f"wg{di}_{gi}_hi")
                    tlo = cp.tile([P, P], bf16, tag=f"wg{di}_{gi}_lo")
                    nc.vector.tensor_copy(thi[:], whi_src[:, gi * P:(gi + 1) * P])
                    nc.vector.tensor_copy(tlo[:], wlo_src[:, gi * P:(gi + 1) * P])
                    wg[(di, gi)] = (thi, tlo)

            # masks batched: one DMA each; dst[p, (r,c)*W + m] <- src[(r,c), p, m]
            lmask_all = cp.tile([P, n_lr * 3 * P], f32)
            nc.scalar.dma_start(
                lmask_all[:],
                AP(d_lmask[:].tensor, 0,
                   [[P, P], [P * P, n_lr * 3], [1, P]]))
            lm_tiles = [[lmask_all[:, (r * 3 + cpt) * P:(r * 3 + cpt + 1) * P]
                         for cpt in range(3)] for r in range(n_lr)]
            vmask_all = cp.tile([P, n_vr * 2 * K], f32)
            nc.scalar.dma_start(
                vmask_all[:],
                AP(d_vmask[:].tensor, 0,
                   [[K, P], [P * K, n_vr * 2], [1, K]]))
            vm_tiles = [[vmask_all[:, (r * 2 + cpt) * K:(r * 2 + cpt + 1) * K]
                         for cpt in range(2)] for r in range(n_vr)]

            iotaB = cp.tile([P, K * K], f32)
            nc.gpsimd.iota(iotaB[:], pattern=[[0, K], [1, K]], base=0,
                           channel_multiplier=0, allow_small_or_imprecise_dtypes=True)
            nc.vector.tensor_scalar(out=iotaB[:], in0=iotaB[:], scalar1=BIG,
                                    scalar2=None, op0=mybir.AluOpType.subtract)

            # token index tiles: tok_sb[p, g] = tok[g*128+p]
            tok_sb = cp.tile([P, NG], i32)
            tokr_sb = cp.tile([P, NG], i32)
            nc.sync.dma_start(tok_sb[:], AP(d_tok[:].tensor, 0, [[1, P], [P, NG]]))
            nc.sync.dma_start(tokr_sb[:], AP(d_tokr[:].tensor, 0, [[1, P], [P, NG]]))

            # ---- gather + transpose -> embT (K=128-padded, with ones row 32) ----
            embT_f = bigp.tile([P, GPAD], f32)
            embT_r = bigp.tile([P, GPAD], f32)
            nc.vector.memset(embT_f[:], 0.0)
            nc.vector.memset(embT_r[:], 0.0)
            nc.vector.memset(embT_f[32:33, :], 1.0)
            nc.vector.memset(embT_r[32:33, :], 1.0)
            for src_idx, dst in ((tok_sb, embT_f), (tokr_sb, embT_r)):
                for g in range(NG):
                    rows = gp.tile([P, EMB], f32, tag="grows")
                    nc.gpsimd.indirect_dma_start(
                        out=rows[:], out_offset=None, in_=d_table[:],
                        in_offset=bass.IndirectOffsetOnAxis(ap=src_idx[:, g:g + 1], axis=0))
                    tp = pp.tile([P, 512], f32, tag="mm")
                    nc.tensor.transpose(out=tp[0:EMB, 0:P], in_=rows[:], identity=ident[:])
                    nc.any.tensor_copy(dst[0:EMB, g * P:(g + 1) * P], tp[0:EMB, 0:P])

            # ---- input projections: pre_all[:, kk, :] kk=(gate*2+dir) ----
            # gate order [i, f, o, g]; dir 0=fwd, 1=bwd(reversed stream)
            pre_all = bigp.tile([P, 8, GPAD], f32)
            for gi in range(4):
                for di, (wih, embT) in enumerate(((wih_f, embT_f), (wih_b, embT_r))):
                    kk = gi * 2 + di
                    for a in range(0, GPAD, 512):
                        n = min(512, GPAD - a)
                        mps = pp.tile([P, 512], f32, tag="mm")
                        nc.tensor.matmul(mps[:, :n], wih[:, gi * P:(gi + 1) * P],
                                         embT[:, a:a + n], start=True, stop=True)
                        nc.any.tensor_copy(pre_all[:, kk, a:a + n], mps[:, :n])

            # ---- LSTM scan: two independent chains (fwd, bwd) interleaved ----
            h_f_store = bigp.tile([P, SLAB], f32)
            h_b_store = bigp.tile([P, SLAB], f32)
            G3 = 3 * NL
            GW = 4 * NL
            chains = []
            for di, store in enumerate((h_f_store, h_b_store)):
                h = st.tile([P, NL], f32, tag=f"h{di}")
                c = st.tile([P, NL], f32, tag=f"c{di}")
                nc.vector.memset(h[:], 0.0)
                nc.vector.memset(c[:], 0.0)
                chains.append({"h": h, "c": c, "store": store, "di": di})

            for s in range(NSTEP):
                for ch in chains:
                    di = ch["di"]
                    h, c, store = ch["h"], ch["c"], ch["store"]
                    if s in LRESET_STEPS:
                        r = LRESET_STEPS.index(s)
                        mh, ah, ac = lm_tiles[r]
                        msl = slice(di * NL, (di + 1) * NL)
                        h2 = st.tile([P, NL], f32, tag=f"h{di}")
                        c2 = st.tile([P, NL], f32, tag=f"c{di}")
                        nc.vector.tensor_tensor(out=h2[:], in0=h[:], in1=mh[:, msl],
                                                op=mybir.AluOpType.mult)
                        nc.vector.tensor_tensor(out=h2[:], in0=h2[:], in1=ah[:, msl],
                                                op=mybir.AluOpType.add)
                        nc.vector.tensor_tensor(out=c2[:], in0=c[:], in1=mh[:, msl],
                                                op=mybir.AluOpType.mult)
                        nc.vector.tensor_tensor(out=c2[:], in0=c2[:], in1=ac[:, msl],
                                                op=mybir.AluOpType.add)
                        h, c = h2, c2

                    h_hi = wk.tile([P, NL], bf16, tag=f"hh{di}")
                    nc.vector.tensor_copy(h_hi[:], h[:])
                    h_lo = wk.tile([P, NL], bf16, tag=f"hl{di}")
                    nc.vector.tensor_tensor(out=h_lo[:], in0=h[:], in1=h_hi[:],
                                            op=mybir.AluOpType.subtract)
                    psz = ppz.tile([P, GW], f32, tag=f"psz{di}")
                    for gi in range(4):
                        thi, tlo = wg[(di, gi)]
                        out_sl = psz[:, gi * NL:(gi + 1) * NL]
                        nc.tensor.matmul(out_sl, thi[:], h_hi[:],
                                         start=True, stop=False)
                        nc.tensor.matmul(out_sl, thi[:], h_lo[:],
                                         start=False, stop=False)
                        nc.tensor.matmul(out_sl, tlo[:], h_hi[:],
                                         start=False, stop=True)
                    # z = psz + pre; pre_all kk = gate*2+di, col = s + 17*lane
                    zsb = wk.tile([P, GW], f32, tag=f"zsb{di}")
                    pre_sl = _ap(pre_all[:], di * GPAD + s, [[2 * GPAD, 4], [L, NL]])
                    nc.vector.tensor_tensor(out=zsb[:], in0=psz[:], in1=pre_sl,
                                            op=mybir.AluOpType.add)
                    sig = wk.tile([P, G3], f32, tag=f"sig{di}")
                    nc.scalar.activation(sig[:], zsb[:, 0:G3],
                                         mybir.ActivationFunctionType.Sigmoid)
                    gg = wk.tile([P, NL], f32, tag=f"gg{di}")
                    nc.scalar.activation(gg[:], zsb[:, G3:GW],
                                         mybir.ActivationFunctionType.Tanh)
                    t1 = wk.tile([P, NL], f32, tag=f"t1{di}")
                    nc.gpsimd.tensor_tensor(out=t1[:], in0=sig[:, 0:NL], in1=gg[:],
                                            op=mybir.AluOpType.mult)
                    c2 = st.tile([P, NL], f32, tag=f"c{di}")
                    nc.vector.tensor_tensor(out=c2[:], in0=sig[:, NL:2 * NL], in1=c[:],
                                            op=mybir.AluOpType.mult)
                    nc.vector.tensor_tensor(out=c2[:], in0=c2[:], in1=t1[:],
                                            op=mybir.AluOpType.add)
                    thc = wk.tile([P, NL], f32, tag=f"thc{di}")
                    nc.scalar.activation(thc[:], c2[:],
                                         mybir.ActivationFunctionType.Tanh)
                    h2 = st.tile([P, NL], f32, tag=f"h{di}")
                    nc.gpsimd.tensor_tensor(out=h2[:], in0=sig[:, 2 * NL:G3],
                                            in1=thc[:], op=mybir.AluOpType.mult)
                    ch["h"], ch["c"] = h2, c2
                    if s >= W:
                        nc.scalar.copy(
                            out=_ap(store[:], s - W, [[L, NL]]), in_=h2[:])

            if debug:
                nc.sync.dma_start(d_hf[:], h_f_store[:])
                nc.sync.dma_start(d_hb[:], h_b_store[:])

            # ---- emissions: feats = W_out @ [h_f; h_b] + b_out, (K, SLAB) ----
            feats_sb = bigp.tile([K, SLAB], f32)
            for a in range(0, SLAB, 512):
                n = min(512, SLAB - a)
                fps = pp.tile([P, 512], f32, tag="mm")
                nc.tensor.matmul(fps[0:K, :n], wout[:, 0:K], h_f_store[:, a:a + n],
                                 start=True, stop=False)
                # h_b_store column c holds global slab position 1087-c
                hb_rev = _ap(h_b_store[:], SLAB - 1 - a, [[-1, n]])
                nc.tensor.matmul(fps[0:K, :n], wout[:, K:2 * K], hb_rev,
                                 start=False, stop=True)
                nc.scalar.activation(feats_sb[:, a:a + n], fps[0:K, :n],
                                     mybir.ActivationFunctionType.Identity,
                                     bias=bout[:, 0:1])
            nc.sync.dma_start(d_feats[:], feats_sb[:])

            # ---- viterbi feats transposes: psum_feats[:, s*K:(s+1)*K] ----
            psum_feats = ppf.tile([P, NVSTEP * K], f32)
            for s in range(NVSTEP):
                # lane j at step s reads feats slab col 32 + 8j + s
                fcols = _ap(feats_sb[:], 32 + s, [[LV, P]])
                nc.tensor.transpose(out=psum_feats[:, s * K:(s + 1) * K],
                                    in_=fcols, identity=ident[0:K, 0:K])

            # ---- viterbi scan ----
            fv = st.tile([P, K], f32, tag="fv")
            nc.vector.memset(fv[:], 0.0)
            fv_store = bigp.tile([P, (LV + 1) * K], f32)
            for s in range(NVSTEP):
                if s in VRESET_STEPS:
                    r = VRESET_STEPS.index(s)
                    mv, av = vm_tiles[r]
                    fv2 = st.tile([P, K], f32, tag="fv")
                    nc.vector.tensor_tensor(out=fv2[:], in0=fv[:], in1=mv[:],
                                            op=mybir.AluOpType.mult)
                    nc.vector.tensor_tensor(out=fv2[:], in0=fv2[:], in1=av[:],
                                            op=mybir.AluOpType.add)
                    fv = fv2
                if s >= WV:
                    nc.vector.tensor_copy(fv_store[:, (s - WV) * K:(s - WV + 1) * K], fv[:])
                S = wk.tile([P, K * K], f32, tag="S")
                fv_b = _ap(fv[:], 0, [[0, K], [1, K]])
                nc.vector.tensor_tensor(out=S[:], in0=fv_b, in1=transb[:],
                                        op=mybir.AluOpType.add)
                M = wk.tile([P, K], f32, tag="M")
                nc.vector.tensor_reduce(out=M[:], in_=_ap(S[:], 0, [[K, K], [1, K]]),
                                        axis=mybir.AxisListType.X, op=mybir.AluOpType.max)
                fv2 = st.tile([P, K], f32, tag="fv")
                nc.vector.tensor_tensor(out=fv2[:], in0=M[:],
                                        in1=psum_feats[:, s * K:(s + 1) * K],
                                        op=mybir.AluOpType.add)
                fv = fv2
            nc.vector.tensor_copy(fv_store[:, LV * K:(LV + 1) * K], fv[:])
            nc.sync.dma_start(d_fv[:], fv_store[:, LV * K:(LV + 1) * K])

            # ---- bulk backpointers ----
            S_all = bigp.tile([P, LV * K * K], f32)
            fv_exp = _ap(fv_store[:], 0, [[K, LV], [0, K], [1, K]])
            tr_exp = _ap(transb[:], 0, [[0, LV], [1, K * K]])
            nc.vector.tensor_tensor(out=S_all[:], in0=fv_exp, in1=tr_exp,
                                    op=mybir.AluOpType.add)
            M_all = bigp.tile([P, LV * K], f32)
            nc.vector.tensor_reduce(out=M_all[:],
                                    in_=_ap(S_all[:], 0, [[K * K, LV], [K, K], [1, K]]),
                                    axis=mybir.AxisListType.X, op=mybir.AluOpType.max)
            EQ = bigp.tile([P, LV * K * K], f32)
            m_exp = _ap(M_all[:], 0, [[1, LV * K], [0, K]])
            nc.vector.tensor_tensor(out=EQ[:], in0=S_all[:], in1=m_exp,
                                    op=mybir.AluOpType.is_equal)
            iota_exp = _ap(iotaB[:], 0, [[0, LV], [1, K * K]])
            nc.vector.tensor_tensor(out=EQ[:], in0=EQ[:], in1=iota_exp,
                                    op=mybir.AluOpType.mult)
            bpf = bigp.tile([P, LV * K], f32)
            nc.vector.tensor_reduce(out=bpf[:],
                                    in_=_ap(EQ[:], 0, [[K * K, LV], [K, K], [1, K]]),
                                    axis=mybir.AxisListType.X, op=mybir.AluOpType.min)
            nc.vector.tensor_scalar(out=bpf[:], in0=bpf[:], scalar1=BIG,
                                    scalar2=None, op0=mybir.AluOpType.add)
            bpi = bigp.tile([P, LV * K], i32)
            nc.vector.tensor_copy(bpi[:], bpf[:])
            nc.sync.dma_start(d_bp[:], bpi[:])

    nc.compile()
    _PROGRAM_CACHE[key] = nc
    return nc


# ---------------- host-side preparation ----------------

def _gate_reorder(wrows):
    """torch gate rows [i,f,g,o] -> our column order [i,f,o,g]; returns (in_dim, 512)."""
    i, f, g, o = np.split(wrows, 4, axis=0)
    return np.concatenate([i.T, f.T, o.T, g.T], axis=1).astype(np.float32)


def _gate_reorder_vec(b):
    i, f, g, o = np.split(b, 4)
    return np.concatenate([i, f, o, g]).astype(np.float32)


def prepare_inputs(sentence, embed_table, w_ih_f, w_hh_f, b_f, w_ih_b, w_hh_b, b_b,
                   W_out, b_out, transitions, h0, c0):
    sent = np.asarray(sentence).astype(np.int64)
    table = np.ascontiguousarray(np.asarray(embed_table, dtype=np.float32))

    import ml_dtypes
    bf = ml_dtypes.bfloat16

    def split_hi_lo(m):
        hi = m.astype(bf)
        lo = (m - hi.astype(np.float32)).astype(bf)
        return np.ascontiguousarray(hi), np.ascontiguousarray(lo)

    whh_fT = np.ascontiguousarray(_gate_reorder(np.asarray(w_hh_f, np.float32)))
    whh_bT = np.ascontiguousarray(_gate_reorder(np.asarray(w_hh_b, np.float32)))
    whh_f_hi, whh_f_lo = split_hi_lo(whh_fT)
    whh_b_hi, whh_b_lo = split_hi_lo(whh_bT)

    def wih_aug(w_ih, b):
        m = np.zeros((P, G4), np.float32)
        m[0:EMB] = _gate_reorder(np.asarray(w_ih, np.float32))
        m[EMB] = _gate_reorder_vec(np.asarray(b, np.float32))
        return m

    wih_f_m = wih_aug(w_ih_f, b_f)
    wih_b_m = wih_aug(w_ih_b, b_b)

    Wo = np.asarray(W_out, np.float32)
    woutT = np.concatenate([Wo[:, 0:H2].T, Wo[:, H2:2 * H2].T], axis=1)
    woutT = np.ascontiguousarray(woutT.astype(np.float32))
    bout = np.asarray(b_out, np.float32).reshape(K, 1).copy()
    trans = np.asarray(transitions, np.float32)
    transb = np.ascontiguousarray(np.tile(trans.reshape(1, K * K), (P, 1)))

    h0 = np.asarray(h0, np.float32)
    c0 = np.asarray(c0, np.float32)
    init_v = np.full(K, NEG_INIT, np.float32)
    init_v[START_IDX] = 0.0

    shared = dict(table=table, whh_f_hi=whh_f_hi, whh_f_lo=whh_f_lo,
                  whh_b_hi=whh_b_hi, whh_b_lo=whh_b_lo, wih_f=wih_f_m,
                  wih_b=wih_b_m, woutT=woutT, bout=bout, transb=transb)

    in_maps = []
    n_lr = len(LRESET_STEPS)
    n_vr = len(VRESET_STEPS)
    for core in range(NCORES):
        base = 1024 * core - 64
        lo = base - W
        gidx = lo + np.arange(GPAD)
        gclamp = np.clip(gidx, 0, T - 1)
        tok = sent[gclamp].astype(np.int32)
        tok[(gidx < 0) | (gidx >= T)] = 0
        # reversed window over the WIN real entries
        tokr = np.zeros(GPAD, np.int32)
        tokr[:WIN] = tok[:WIN][::-1]

        lmask = np.zeros((n_lr, 3, P, P), np.float32)
        lmask[:, 0] = 1.0
        for s, l in LRESET_F:
            if core == 0:
                r = LRESET_STEPS.index(s)
                lmask[r, 0, :, l] = 0.0
                lmask[r, 1, :, l] = h0[0]
                lmask[r, 2, :, l] = c0[0]
        for s, j in LRESET_B:
            if core == NCORES - 1:
                r = LRESET_STEPS.index(s)
                lmask[r, 0, :, NL + j] = 0.0
                lmask[r, 1, :, NL + j] = h0[1]
                lmask[r, 2, :, NL + j] = c0[1]

        vmask = np.zeros((n_vr, 2, P, K), np.float32)
        vmask[:, 0] = 1.0
        if core == 0:
            for s, j in VRESET:
                r = VRESET_STEPS.index(s)
                vmask[r, 0, j, :] = 0.0
                vmask[r, 1, j, :] = init_v
        m = dict(shared)
        m.update(tok=tok, tokr=tokr, lmask=lmask, vmask=vmask)
        in_maps.append(m)
    return in_maps


def postprocess(results, transitions):
    trans = np.asarray(transitions, np.float64)
    bp_all = np.zeros((T, K), np.int64)
    feats_all = np.zeros((T, K), np.float64)
    for core in range(NCORES):
        bp = np.asarray(results[core]["bp"]).reshape(P, LV, K)
        for j in range(P):
            t0 = 1024 * core + LV * j
            bp_all[t0:t0 + LV] = bp[j]
        feats = np.asarray(results[core]["feats"])  # (K, SLAB)
        feats_all[1024 * core:1024 * (core + 1)] = feats[:, 64:64 + 1024].T
    fv_final = np.asarray(results[NCORES - 1]["fv"])[P - 1].astype(np.float64)
    terminal = fv_final + trans[END_IDX]
    best = int(np.argmax(terminal))
    path = np.zeros(T, np.int64)
    cur = best
    for t in range(T - 1, -1, -1):
        path[t] = cur
        cur = bp_all[t, cur]
    # replicate the reference's f32 sequential accumulation along the path
    tr32 = np.asarray(transitions, np.float32)
    f32_all = feats_all.astype(np.float32)
    s = np.float32(tr32[path[0], START_IDX])
    s = np.float32(s + f32_all[0, path[0]])
    for t in range(1, T):
        s = np.float32(s + tr32[path[t], path[t - 1]])
        s = np.float32(s + f32_all[t, path[t]])
    s = np.float32(s + tr32[END_IDX, path[-1]])
    return s, path.astype(np.int32)


def kernel(sentence, embed_table, w_ih_f, w_hh_f, b_f, w_ih_b, w_hh_b, b_b,
           W_out, b_out, transitions, h0, c0):
    from concourse.bass_utils import run_bass_kernel_spmd

    nc = build_program(debug=False)
    in_maps = prepare_inputs(sentence, embed_table, w_ih_f, w_hh_f, b_f,
                             w_ih_b, w_hh_b, b_b, W_out, b_out, transitions, h0, c0)
    res = run_bass_kernel_spmd(nc, in_maps, core_ids=list(range(NCORES)))
    return postprocess(res.results, transitions)


# revision 17
# speedup vs baseline: 1.0758x; 1.0758x over previous
"""BiLSTM-CRF Trainium2 kernel (8-core SPMD, no collectives).

Strategy: chunk-parallel scans with warmup. The LSTM forget-gate contraction
(~0.6/step for these weights) makes a chunk that warms up for W steps from a
wrong initial state converge to the exact sequential state; chunks that cross
the true sequence boundary get an exact masked state override instead. The
same idea parallelizes the Viterbi forward pass (max-plus coalescing). Each
core independently processes a 1088-token slab (1024 tokens + halo for the
Viterbi warmup): embedding gather (indirect DMA) -> input projections (PE)
-> 57-step vectorized BiLSTM scan over 128 chunk-lanes -> emission matmul ->
40-step vectorized Viterbi with bulk backpointer extraction. The host only
does the O(T) backtrace pointer-chase and the scalar score sum.
"""

import numpy as np

import concourse.bass as bass
import concourse.tile as tile
from concourse import bacc, mybir
from concourse.bass import AP
from concourse.masks import make_identity

f32 = mybir.dt.float32
i32 = mybir.dt.int32
bf16 = mybir.dt.bfloat16

# problem constants (hardcoded per contract)
VOCAB = 1_000_000
EMB = 32
H2 = 128
G4 = 4 * H2
K = 11
START_IDX = 9
END_IDX = 10
T = 8192
NCORES = 8

# geometry
P = 128
NL = 64            # lanes per direction per core
L = 17             # real tokens per LSTM lane
W = 32             # LSTM warmup steps
SLAB = 1088        # NL * L, real tokens per core (incl. viterbi halo)
WIN = SLAB + 2 * W  # gathered token window (1168), padded to GPAD
GPAD = 1280        # 10 gather groups of 128
NSTEP = W + L      # 57
LV = 8             # real tokens per viterbi lane
WV = 32            # viterbi warmup steps
NVSTEP = WV + LV   # 40
BIG = 2048.0
NEG_INIT = -10000.0

# reset (exact-init override) steps
LRESET_F = [(W + 64 - 17 * l, l) for l in range(8) if 0 <= W + 64 - 17 * l <= NSTEP - 1]
LRESET_B = [(W - 17 * j, j) for j in range(8) if 0 <= W - 17 * j <= NSTEP - 1]
LRESET_STEPS = sorted({s for s, _ in LRESET_F} | {s for s, _ in LRESET_B})
VRESET = [(WV - LV * j, j) for j in range(WV // LV + 1) if WV - LV * j >= 0]
VRESET_STEPS = sorted({s for s, _ in VRESET})

_PROGRAM_CACHE = {}


def _ap(t_ap, extra_offset, dims):
    return AP(t_ap.tensor, t_ap.offset + extra_offset, [t_ap.ap[0]] + dims)


def build_program(debug=False):
    key = bool(debug)
    if key in _PROGRAM_CACHE:
        return _PROGRAM_CACHE[key]

    nc = bacc.Bacc("TRN2", target_bir_lowering=False, debug=True)

    d_table = nc.dram_tensor("table", (VOCAB, EMB), f32, kind="ExternalInput")
    d_tok = nc.dram_tensor("tok", (GPAD,), i32, kind="ExternalInput")
    d_tokr = nc.dram_tensor("tokr", (GPAD,), i32, kind="ExternalInput")
    d_whh_f_hi = nc.dram_tensor("whh_f_hi", (P, G4), bf16, kind="ExternalInput")
    d_whh_f_lo = nc.dram_tensor("whh_f_lo", (P, G4), bf16, kind="ExternalInput")
    d_whh_b_hi = nc.dram_tensor("whh_b_hi", (P, G4), bf16, kind="ExternalInput")
    d_whh_b_lo = nc.dram_tensor("whh_b_lo", (P, G4), bf16, kind="ExternalInput")
    d_wih_f = nc.dram_tensor("wih_f", (P, G4), f32, kind="ExternalInput")
    d_wih_b = nc.dram_tensor("wih_b", (P, G4), f32, kind="ExternalInput")
    d_wout = nc.dram_tensor("woutT", (P, 2 * K), f32, kind="ExternalInput")
    d_bout = nc.dram_tensor("bout", (K, 1), f32, kind="ExternalInput")
    d_transb = nc.dram_tensor("transb", (P, K * K), f32, kind="ExternalInput")
    n_lr = len(LRESET_STEPS)
    n_vr = len(VRESET_STEPS)
    d_lmask = nc.dram_tensor("lmask", (n_lr, 3, P, P), f32, kind="ExternalInput")
    d_vmask = nc.dram_tensor("vmask", (n_vr, 2, P, K), f32, kind="ExternalInput")

    d_bp = nc.dram_tensor("bp", (P, LV * K), i32, kind="ExternalOutput")
    d_fv = nc.dram_tensor("fv", (P, K), f32, kind="ExternalOutput")
    d_feats = nc.dram_tensor("feats", (K, SLAB), f32, kind="ExternalOutput")
    if debug:
        d_hf = nc.dram_tensor("hf", (P, SLAB), f32, kind="ExternalOutput")
        d_hb = nc.dram_tensor("hb", (P, SLAB), f32, kind="ExternalOutput")

    NG = GPAD // P  # gather groups

    with tile.TileContext(nc) as tc:
        with (
            tc.tile_pool(name="const", bufs=1) as cp,
            tc.tile_pool(name="big", bufs=1) as bigp,
            tc.tile_pool(name="gat", bufs=3) as gp,
            tc.tile_pool(name="state", bufs=3) as st,
            tc.tile_pool(name="work", bufs=3) as wk,
            tc.tile_pool(name="ps", bufs=2, space="PSUM") as pp,
            tc.tile_pool(name="psz", bufs=2, space="PSUM") as ppz,
            tc.tile_pool(name="psf", bufs=1, space="PSUM") as ppf,
        ):
            # ---- constants ----
            ident = cp.tile([P, P], f32)
            make_identity(nc, ident[:])
            whh_f_hi = cp.tile([P, G4], bf16)
            whh_f_lo = cp.tile([P, G4], bf16)
            whh_b_hi = cp.tile([P, G4], bf16)
            whh_b_lo = cp.tile([P, G4], bf16)
            wih_f = cp.tile([P, G4], f32)
            wih_b = cp.tile([P, G4], f32)
            wout = cp.tile([P, 2 * K], f32)
            bout = cp.tile([K, 1], f32)
            transb = cp.tile([P, K * K], f32)
            nc.sync.dma_start(whh_f_hi[:], d_whh_f_hi[:])
            nc.sync.dma_start(whh_f_lo[:], d_whh_f_lo[:])
            nc.sync.dma_start(whh_b_hi[:], d_whh_b_hi[:])
            nc.sync.dma_start(whh_b_lo[:], d_whh_b_lo[:])
            nc.sync.dma_start(wih_f[:], d_wih_f[:])
            nc.sync.dma_start(wih_b[:], d_wih_b[:])
            nc.sync.dma_start(wout[:], d_wout[:])
            nc.sync.dma_start(bout[:], d_bout[:])
            nc.sync.dma_start(transb[:], d_transb[:])

            # contiguous per-gate weight tiles so LDWEIGHTS can use FWL
            wg = {}
            for di, (whi_src, wlo_src) in enumerate(((whh_f_hi, whh_f_lo),
                                                    (whh_b_hi, whh_b_lo))):
                for gi in range(4):
                    thi = cp.tile([P, P], bf16, tag=f"wg{di}_{gi}_hi")
                    tlo = cp.tile([P, P], bf16, tag=f"wg{di}_{gi}_lo")
                    nc.vector.tensor_copy(thi[:], whi_src[:, gi * P:(gi + 1) * P])
                    nc.vector.tensor_copy(tlo[:], wlo_src[:, gi * P:(gi + 1) * P])
                    wg[(di, gi)] = (thi, tlo)

            # masks batched: one DMA each; dst[p, (r,c)*W + m] <- src[(r,c), p, m]
            lmask_all = cp.tile([P, n_lr * 3 * P], f32)
            nc.scalar.dma_start(
                lmask_all[:],
                AP(d_lmask[:].tensor, 0,
                   [[P, P], [P * P, n_lr * 3], [1, P]]))
            lm_tiles = [[lmask_all[:, (r * 3 + cpt) * P:(r * 3 + cpt + 1) * P]
                         for cpt in range(3)] for r in range(n_lr)]
            vmask_all = cp.tile([P, n_vr * 2 * K], f32)
            nc.scalar.dma_start(
                vmask_all[:],
                AP(d_vmask[:].tensor, 0,
                   [[K, P], [P * K, n_vr * 2], [1, K]]))
            vm_tiles = [[vmask_all[:, (r * 2 + cpt) * K:(r * 2 + cpt + 1) * K]
                         for cpt in range(2)] for r in range(n_vr)]

            iotaB = cp.tile([P, K * K], f32)
            nc.gpsimd.iota(iotaB[:], pattern=[[0, K], [1, K]], base=0,
                           channel_multiplier=0, allow_small_or_imprecise_dtypes=True)
            nc.vector.tensor_scalar(out=iotaB[:], in0=iotaB[:], scalar1=BIG,
                                    scalar2=None, op0=mybir.AluOpType.subtract)

            # token index tiles: tok_sb[p, g] = tok[g*128+p]
            tok_sb = cp.tile([P, NG], i32)
            tokr_sb = cp.tile([P, NG], i32)
            nc.sync.dma_start(tok_sb[:], AP(d_tok[:].tensor, 0, [[1, P], [P, NG]]))
            nc.sync.dma_start(tokr_sb[:], AP(d_tokr[:].tensor, 0, [[1, P], [P, NG]]))

            # ---- gather + transpose -> embT (K=128-padded, with ones row 32) ----
            embT_f = bigp.tile([P, GPAD], f32)
            embT_r = bigp.tile([P, GPAD], f32)
            nc.vector.memset(embT_f[:], 0.0)
            nc.vector.memset(embT_r[:], 0.0)
            nc.vector.memset(embT_f[32:33, :], 1.0)
            nc.vector.memset(embT_r[32:33, :], 1.0)
            for src_idx, dst in ((tok_sb, embT_f), (tokr_sb, embT_r)):
                for g in range(NG):
                    rows = gp.tile([P, EMB], f32, tag="grows")
                    nc.gpsimd.indirect_dma_start(
                        out=rows[:], out_offset=None, in_=d_table[:],
                        in_offset=bass.IndirectOffsetOnAxis(ap=src_idx[:, g:g + 1], axis=0))
                    tp = pp.tile([P, 512], f32, tag="mm")
                    nc.tensor.transpose(out=tp[0:EMB, 0:P], in_=rows[:], identity=ident[:])
                    nc.any.tensor_copy(dst[0:EMB, g * P:(g + 1) * P], tp[0:EMB, 0:P])

            # ---- input projections: pre_all[:, kk, :] kk=(gate*2+dir) ----
            # gate order [i, f, o, g]; dir 0=fwd, 1=bwd(reversed stream)
            pre_all = bigp.tile([P, 8, GPAD], f32)
            for gi in range(4):
                for di, (wih, embT) in enumerate(((wih_f, embT_f), (wih_b, embT_r))):
                    kk = gi * 2 + di
                    for a in range(0, GPAD, 512):
                        n = min(512, GPAD - a)
                        mps = pp.tile([P, 512], f32, tag="mm")
                        nc.tensor.matmul(mps[:, :n], wih[:, gi * P:(gi + 1) * P],
                                         embT[:, a:a + n], start=True, stop=True)
                        nc.any.tensor_copy(pre_all[:, kk, a:a + n], mps[:, :n])

            # ---- LSTM scan: two independent chains (fwd, bwd) interleaved ----
            h_f_store = bigp.tile([P, SLAB], f32)
            h_b_store = bigp.tile([P, SLAB], f32)
            G3 = 3 * NL
            GW = 4 * NL
            chains = []
            for di, store in enumerate((h_f_store, h_b_store)):
                h = st.tile([P, NL], f32, tag=f"h{di}")
                c = st.tile([P, NL], f32, tag=f"c{di}")
                nc.vector.memset(h[:], 0.0)
                nc.vector.memset(c[:], 0.0)
                chains.append({"h": h, "c": c, "store": store, "di": di})

            for s in range(NSTEP):
                # phase 1: resets + h splits (both chains)
                for ch in chains:
                    di = ch["di"]
                    h, c = ch["h"], ch["c"]
                    if s in LRESET_STEPS:
                        r = LRESET_STEPS.index(s)
                        mh, ah, ac = lm_tiles[r]
                        msl = slice(di * NL, (di + 1) * NL)
                        h2 = st.tile([P, NL], f32, tag=f"h{di}")
                        c2 = st.tile([P, NL], f32, tag=f"c{di}")
                        nc.vector.tensor_tensor(out=h2[:], in0=h[:], in1=mh[:, msl],
                                                op=mybir.AluOpType.mult)
                        nc.vector.tensor_tensor(out=h2[:], in0=h2[:], in1=ah[:, msl],
                                                op=mybir.AluOpType.add)
                        nc.vector.tensor_tensor(out=c2[:], in0=c[:], in1=mh[:, msl],
                                                op=mybir.AluOpType.mult)
                        nc.vector.tensor_tensor(out=c2[:], in0=c2[:], in1=ac[:, msl],
                                                op=mybir.AluOpType.add)
                        h, c = h2, c2
                        ch["h"], ch["c"] = h, c
                    h_hi = wk.tile([P, NL], bf16, tag=f"hh{di}")
                    nc.scalar.copy(out=h_hi[:], in_=h[:])
                    h_lo = wk.tile([P, NL], bf16, tag=f"hl{di}")
                    nc.vector.tensor_tensor(out=h_lo[:], in0=h[:], in1=h_hi[:],
                                            op=mybir.AluOpType.subtract)
                    ch["h_hi"], ch["h_lo"] = h_hi, h_lo
                    ch["psz"] = ppz.tile([P, GW], f32, tag=f"psz{di}", name=f"psz{di}")
                # phase 2: matmuls, chains interleaved so accumulate-drains hide
                for gi in range(4):
                    for term in range(3):
                        for ch in chains:
                            di = ch["di"]
                            thi, tlo = wg[(di, gi)]
                            lhs = thi if term < 2 else tlo
                            rhs = ch["h_hi"] if term != 1 else ch["h_lo"]
                            nc.tensor.matmul(ch["psz"][:, gi * NL:(gi + 1) * NL],
                                             lhs[:], rhs[:],
                                             start=(term == 0), stop=(term == 2))
                # phase 3: per-chain elementwise tails
                for ch in chains:
                    di = ch["di"]
                    h, c, store, psz = ch["h"], ch["c"], ch["store"], ch["psz"]
                    zsb = wk.tile([P, GW], f32, tag=f"zsb{di}")
                    pre_sl = _ap(pre_all[:], di * GPAD + s, [[2 * GPAD, 4], [L, NL]])
                    nc.vector.tensor_tensor(out=zsb[:], in0=psz[:], in1=pre_sl,
                                            op=mybir.AluOpType.add)
                    sig = wk.tile([P, G3], f32, tag=f"sig{di}")
                    nc.scalar.activation(sig[:], zsb[:, 0:G3],
                                         mybir.ActivationFunctionType.Sigmoid)
                    gg = wk.tile([P, NL], f32, tag=f"gg{di}")
                    nc.scalar.activation(gg[:], zsb[:, G3:GW],
                                         mybir.ActivationFunctionType.Tanh)
                    t1 = wk.tile([P, NL], f32, tag=f"t1{di}")
                    nc.vector.tensor_tensor(out=t1[:], in0=sig[:, 0:NL], in1=gg[:],
                                            op=mybir.AluOpType.mult)
                    c2 = st.tile([P, NL], f32, tag=f"c{di}")
                    nc.vector.tensor_tensor(out=c2[:], in0=sig[:, NL:2 * NL], in1=c[:],
                                            op=mybir.AluOpType.mult)
                    nc.vector.tensor_tensor(out=c2[:], in0=c2[:], in1=t1[:],
                                            op=mybir.AluOpType.add)
                    thc = wk.tile([P, NL], f32, tag=f"thc{di}")
                    nc.scalar.activation(thc[:], c2[:],
                                         mybir.ActivationFunctionType.Tanh)
                    h2 = st.tile([P, NL], f32, tag=f"h{di}")
                    nc.vector.tensor_tensor(out=h2[:], in0=sig[:, 2 * NL:G3],
                                            in1=thc[:], op=mybir.AluOpType.mult)
                    ch["h"], ch["c"] = h2, c2
                    if s >= W:
                        nc.scalar.copy(
                            out=_ap(store[:], s - W, [[L, NL]]), in_=h2[:])

            if debug:
                nc.sync.dma_start(d_hf[:], h_f_store[:])
                nc.sync.dma_start(d_hb[:], h_b_store[:])

            # ---- emissions: feats = W_out @ [h_f; h_b] + b_out, (K, SLAB) ----
            feats_sb = bigp.tile([K, SLAB], f32)
            for a in range(0, SLAB, 512):
                n = min(512, SLAB - a)
                fps = pp.tile([P, 512], f32, tag="mm")
                nc.tensor.matmul(fps[0:K, :n], wout[:, 0:K], h_f_store[:, a:a + n],
                                 start=True, stop=False)
                # h_b_store column c holds global slab position 1087-c
                hb_rev = _ap(h_b_store[:], SLAB - 1 - a, [[-1, n]])
                nc.tensor.matmul(fps[0:K, :n], wout[:, K:2 * K], hb_rev,
                                 start=False, stop=True)
                nc.scalar.activation(feats_sb[:, a:a + n], fps[0:K, :n],
                                     mybir.ActivationFunctionType.Identity,
                                     bias=bout[:, 0:1])
            nc.sync.dma_start(d_feats[:], feats_sb[:])

            # ---- viterbi feats transposes: psum_feats[:, s*K:(s+1)*K] ----
            psum_feats = ppf.tile([P, NVSTEP * K], f32)
            for s in range(NVSTEP):
                # lane j at step s reads feats slab col 32 + 8j + s
                fcols = _ap(feats_sb[:], 32 + s, [[LV, P]])
                nc.tensor.transpose(out=psum_feats[:, s * K:(s + 1) * K],
                                    in_=fcols, identity=ident[0:K, 0:K])

            # ---- viterbi scan ----
            fv = st.tile([P, K], f32, tag="fv")
            nc.vector.memset(fv[:], 0.0)
            fv_store = bigp.tile([P, (LV + 1) * K], f32)
            for s in range(NVSTEP):
                if s in VRESET_STEPS:
                    r = VRESET_STEPS.index(s)
                    mv, av = vm_tiles[r]
                    fv2 = st.tile([P, K], f32, tag="fv")
                    nc.vector.tensor_tensor(out=fv2[:], in0=fv[:], in1=mv[:],
                                            op=mybir.AluOpType.mult)
                    nc.vector.tensor_tensor(out=fv2[:], in0=fv2[:], in1=av[:],
                                            op=mybir.AluOpType.add)
                    fv = fv2
                if s >= WV:
                    nc.vector.tensor_copy(fv_store[:, (s - WV) * K:(s - WV + 1) * K], fv[:])
                S = wk.tile([P, K * K], f32, tag="S")
                fv_b = _ap(fv[:], 0, [[0, K], [1, K]])
                nc.vector.tensor_tensor(out=S[:], in0=fv_b, in1=transb[:],
                                        op=mybir.AluOpType.add)
                M = wk.tile([P, K], f32, tag="M")
                nc.vector.tensor_reduce(out=M[:], in_=_ap(S[:], 0, [[K, K], [1, K]]),
                                        axis=mybir.AxisListType.X, op=mybir.AluOpType.max)
                fv2 = st.tile([P, K], f32, tag="fv")
                nc.vector.tensor_tensor(out=fv2[:], in0=M[:],
                                        in1=psum_feats[:, s * K:(s + 1) * K],
                                        op=mybir.AluOpType.add)
                fv = fv2
            nc.vector.tensor_copy(fv_store[:, LV * K:(LV + 1) * K], fv[:])
            nc.sync.dma_start(d_fv[:], fv_store[:, LV * K:(LV + 1) * K])

            # ---- bulk backpointers ----
            S_all = bigp.tile([P, LV * K * K], f32)
            fv_exp = _ap(fv_store[:], 0, [[K, LV], [0, K], [1, K]])
            tr_exp = _ap(transb[:], 0, [[0, LV], [1, K * K]])
            nc.vector.tensor_tensor(out=S_all[:], in0=fv_exp, in1=tr_exp,
                                    op=mybir.AluOpType.add)
            M_all = bigp.tile([P, LV * K], f32)
            nc.vector.tensor_reduce(out=M_all[:],
                                    in_=_ap(S_all[:], 0, [[K * K, LV], [K, K], [1, K]]),
                                    axis=mybir.AxisListType.X, op=mybir.AluOpType.max)
            EQ = bigp.tile([P, LV * K * K], f32)
            m_exp = _ap(M_all[:], 0, [[1, LV * K], [0, K]])
            nc.vector.tensor_tensor(out=EQ[:], in0=S_all[:], in1=m_exp,
                                    op=mybir.AluOpType.is_equal)
            iota_exp = _ap(iotaB[:], 0, [[0, LV], [1, K * K]])
            nc.vector.tensor_tensor(out=EQ[:], in0=EQ[:], in1=iota_exp,
                                    op=mybir.AluOpType.mult)
            bpf = bigp.tile([P, LV * K], f32)
            nc.vector.tensor_reduce(out=bpf[:],
                                    in_=_ap(EQ[:], 0, [[K * K, LV], [K, K], [1, K]]),
                                    axis=mybir.AxisListType.X, op=mybir.AluOpType.min)
            nc.vector.tensor_scalar(out=bpf[:], in0=bpf[:], scalar1=BIG,
                                    scalar2=None, op0=mybir.AluOpType.add)
            bpi = bigp.tile([P, LV * K], i32)
            nc.vector.tensor_copy(bpi[:], bpf[:])
            nc.sync.dma_start(d_bp[:], bpi[:])

    nc.compile()
    _PROGRAM_CACHE[key] = nc
    return nc


# ---------------- host-side preparation ----------------

def _gate_reorder(wrows):
    """torch gate rows [i,f,g,o] -> our column order [i,f,o,g]; returns (in_dim, 512)."""
    i, f, g, o = np.split(wrows, 4, axis=0)
    return np.concatenate([i.T, f.T, o.T, g.T], axis=1).astype(np.float32)


def _gate_reorder_vec(b):
    i, f, g, o = np.split(b, 4)
    return np.concatenate([i, f, o, g]).astype(np.float32)


def prepare_inputs(sentence, embed_table, w_ih_f, w_hh_f, b_f, w_ih_b, w_hh_b, b_b,
                   W_out, b_out, transitions, h0, c0):
    sent = np.asarray(sentence).astype(np.int64)
    table = np.ascontiguousarray(np.asarray(embed_table, dtype=np.float32))

    import ml_dtypes
    bf = ml_dtypes.bfloat16

    def split_hi_lo(m):
        hi = m.astype(bf)
        lo = (m - hi.astype(np.float32)).astype(bf)
        return np.ascontiguousarray(hi), np.ascontiguousarray(lo)

    whh_fT = np.ascontiguousarray(_gate_reorder(np.asarray(w_hh_f, np.float32)))
    whh_bT = np.ascontiguousarray(_gate_reorder(np.asarray(w_hh_b, np.float32)))
    whh_f_hi, whh_f_lo = split_hi_lo(whh_fT)
    whh_b_hi, whh_b_lo = split_hi_lo(whh_bT)

    def wih_aug(w_ih, b):
        m = np.zeros((P, G4), np.float32)
        m[0:EMB] = _gate_reorder(np.asarray(w_ih, np.float32))
        m[EMB] = _gate_reorder_vec(np.asarray(b, np.float32))
        return m

    wih_f_m = wih_aug(w_ih_f, b_f)
    wih_b_m = wih_aug(w_ih_b, b_b)

    Wo = np.asarray(W_out, np.float32)
    woutT = np.concatenate([Wo[:, 0:H2].T, Wo[:, H2:2 * H2].T], axis=1)
    woutT = np.ascontiguousarray(woutT.astype(np.float32))
    bout = np.asarray(b_out, np.float32).reshape(K, 1).copy()
    trans = np.asarray(transitions, np.float32)
    transb = np.ascontiguousarray(np.tile(trans.reshape(1, K * K), (P, 1)))

    h0 = np.asarray(h0, np.float32)
    c0 = np.asarray(c0, np.float32)
    init_v = np.full(K, NEG_INIT, np.float32)
    init_v[START_IDX] = 0.0

    shared = dict(table=table, whh_f_hi=whh_f_hi, whh_f_lo=whh_f_lo,
                  whh_b_hi=whh_b_hi, whh_b_lo=whh_b_lo, wih_f=wih_f_m,
                  wih_b=wih_b_m, woutT=woutT, bout=bout, transb=transb)

    in_maps = []
    n_lr = len(LRESET_STEPS)
    n_vr = len(VRESET_STEPS)
    for core in range(NCORES):
        base = 1024 * core - 64
        lo = base - W
        gidx = lo + np.arange(GPAD)
        gclamp = np.clip(gidx, 0, T - 1)
        tok = sent[gclamp].astype(np.int32)
        tok[(gidx < 0) | (gidx >= T)] = 0
        # reversed window over the WIN real entries
        tokr = np.zeros(GPAD, np.int32)
        tokr[:WIN] = tok[:WIN][::-1]

        lmask = np.zeros((n_lr, 3, P, P), np.float32)
        lmask[:, 0] = 1.0
        for s, l in LRESET_F:
            if core == 0:
                r = LRESET_STEPS.index(s)
                lmask[r, 0, :, l] = 0.0
                lmask[r, 1, :, l] = h0[0]
                lmask[r, 2, :, l] = c0[0]
        for s, j in LRESET_B:
            if core == NCORES - 1:
                r = LRESET_STEPS.index(s)
                lmask[r, 0, :, NL + j] = 0.0
                lmask[r, 1, :, NL + j] = h0[1]
                lmask[r, 2, :, NL + j] = c0[1]

        vmask = np.zeros((n_vr, 2, P, K), np.float32)
        vmask[:, 0] = 1.0
        if core == 0:
            for s, j in VRESET:
                r = VRESET_STEPS.index(s)
                vmask[r, 0, j, :] = 0.0
                vmask[r, 1, j, :] = init_v
        m = dict(shared)
        m.update(tok=tok, tokr=tokr, lmask=lmask, vmask=vmask)
        in_maps.append(m)
    return in_maps


def postprocess(results, transitions):
    trans = np.asarray(transitions, np.float64)
    bp_all = np.zeros((T, K), np.int64)
    feats_all = np.zeros((T, K), np.float64)
    for core in range(NCORES):
        bp = np.asarray(results[core]["bp"]).reshape(P, LV, K)
        for j in range(P):
            t0 = 1024 * core + LV * j
            bp_all[t0:t0 + LV] = bp[j]
        feats = np.asarray(results[core]["feats"])  # (K, SLAB)
        feats_all[1024 * core:1024 * (core + 1)] = feats[:, 64:64 + 1024].T
    fv_final = np.asarray(results[NCORES - 1]["fv"])[P - 1].astype(np.float64)
    terminal = fv_final + trans[END_IDX]
    best = int(np.argmax(terminal))
    path = np.zeros(T, np.int64)
    cur = best
    for t in range(T - 1, -1, -1):
        path[t] = cur
        cur = bp_all[t, cur]
    # replicate the reference's f32 sequential accumulation along the path
    tr32 = np.asarray(transitions, np.float32)
    f32_all = feats_all.astype(np.float32)
    s = np.float32(tr32[path[0], START_IDX])
    s = np.float32(s + f32_all[0, path[0]])
    for t in range(1, T):
        s = np.float32(s + tr32[path[t], path[t - 1]])
        s = np.float32(s + f32_all[t, path[t]])
    s = np.float32(s + tr32[END_IDX, path[-1]])
    return s, path.astype(np.int32)


def kernel(sentence, embed_table, w_ih_f, w_hh_f, b_f, w_ih_b, w_hh_b, b_b,
           W_out, b_out, transitions, h0, c0):
    from concourse.bass_utils import run_bass_kernel_spmd

    nc = build_program(debug=False)
    in_maps = prepare_inputs(sentence, embed_table, w_ih_f, w_hh_f, b_f,
                             w_ih_b, w_hh_b, b_b, W_out, b_out, transitions, h0, c0)
    res = run_bass_kernel_spmd(nc, in_maps, core_ids=list(range(NCORES)))
    return postprocess(res.results, transitions)
